# revision 8
# baseline (speedup 1.0000x reference)
"""Boolean OR-matmul kernel for Trainium2 (8 NeuronCores).

out[b, i] = OR_j (x[b, j] AND w[i, j])  ==  (x_f32 @ w.T_f32) > 0

Strategy:
- Shard bit_weights rows (layer_size 8192) across 8 cores -> 1024 rows/core,
  replicate x. No cross-core reduction needed; host concatenates column
  blocks of the output.
- Encode bools as fp8_e4m3 0.0/1.0 (bit pattern 0x38 == 1.0). Products are
  exactly 0/1, PSUM accumulates fp32 (counts <= 8192 < 2^24, exact), so
  (count > 0) is exact.
- Host pre-transposes both operands to put the contraction dim (in_features
  D) on the SBUF partition axis: xT (D, B), wT (D, Lshard). This makes every
  DMA a clean 2D/3D strided pattern with >=512B contiguous runs.
- PE does fp8 DoubleRow matmuls (K=256 per instruction), which the trace
  shows running at the true HW peak (216 ns per N=512 MM, LDWEIGHTS fully
  hidden). All remaining headroom is at the edges:
  * graduated fine-grained W/X chunks with W on the sync queue and X on the
    scalar-engine HWDGE queue, so the first matmul gates on ~200 KB instead
    of ~400 KB and slab-0 never outruns the DMA wave;
  * a short burst of dummy bf16 matmuls issued while the first chunks are
    still in flight pre-triggers the PE HAM clock-gate (cold 1.2 GHz ->
    warm 2.4 GHz) so the real stream starts warm;
  * slabs >= 1 load X as one whole-slab DMA (prefetch is ~50 us ahead).
- DVE thresholds PSUM fp32 -> uint8 0/1 via is_gt, DMA out.
"""

import sys

for _p in ("/opt/trn_rl_repo",):
    if _p not in sys.path:
        sys.path.insert(0, _p)

import numpy as np
import ml_dtypes

import concourse.bass as bass
import concourse.tile as tile
from concourse import bacc, mybir
from concourse.bass_utils import run_bass_kernel_spmd

P = 128          # SBUF partitions / PE contraction per k-subtile
N_CORES = 8

# Full problem shapes (hardcoded per harness contract)
BATCH = 4096
IN_DIM = 8192
LAYER_SIZE = 8192
L_SHARD = LAYER_SIZE // N_CORES  # 1024

N_WARM = 16      # dummy matmuls to pre-warm the PE HAM clock gate


def build_nc(B, D, L, b_slab=512, n_free=512):
    """Build the per-core Bass program.

    Per-core inputs : xT (D, B) fp8e4, wT (D, L) fp8e4
    Per-core output : out (B, L) uint8 (0/1)
    """
    assert D % (2 * P) == 0 and B % P == 0
    assert L % n_free == 0
    KSUB = D // P               # k-subtiles of 128
    NL = L // n_free            # l tiles
    assert B % b_slab == 0
    slabs = [b_slab] * (B // b_slab)
    offsets = [sum(slabs[:i]) for i in range(len(slabs))]

    nc = bacc.Bacc(None, target_bir_lowering=False, debug=False)
    xT = nc.dram_tensor("xT", [D, B], mybir.dt.float8e4, kind="ExternalInput")
    wT = nc.dram_tensor("wT", [D, L], mybir.dt.float8e4, kind="ExternalInput")
    out = nc.dram_tensor("out", [B, L], mybir.dt.uint8, kind="ExternalOutput")

    xT_r = xT.rearrange("(nk p) b -> p nk b", p=P)   # [128, KSUB, B]
    wT_r = wT.rearrange("(nk p) l -> p nk l", p=P)   # [128, KSUB, L]

    # Slab-0 is DMA-paced: chunk boundaries graduated so the first matmul
    # gates on a single k-subtile and the catch-up granularity stays fine
    # while the 12 MB W+X0 preload is in flight.
    bounds = sorted(
        {b for b in (0, 2, 4, 6, 8, 12, 16, 24, 32, 40, 48, 56) if b < KSUB}
        | {KSUB}
    )
    chunks = list(zip(bounds[:-1], bounds[1:]))  # [(lo, hi), ...]
    ks2chunk = {}
    for ci, (lo, hi) in enumerate(chunks):
        for ks in range(lo, hi):
            ks2chunk[ks] = (ci, ks - lo)

    with tile.TileContext(nc) as tc:
        with (
            tc.tile_pool(name="warm", bufs=1) as warmpool,
            tc.tile_pool(name="wpool", bufs=1) as wpool,
            tc.tile_pool(name="x0pool", bufs=1) as x0pool,
            tc.tile_pool(name="xpool", bufs=2) as xpool,
            tc.tile_pool(name="opool", bufs=4) as opool,
            tc.tile_pool(name="psum", bufs=8, space="PSUM") as pspool,
        ):
            # --- HAM pre-warm: cheap bf16 matmuls on a memset scratch tile
            # run while the first W/X chunks are still in flight, so the
            # PE clock gate opens before the real stream begins.
            warm_sb = warmpool.tile([P, 256], mybir.dt.bfloat16, name="warm")
            nc.gpsimd.memset(warm_sb[:], 0.0)
            ps_warm = pspool.tile([P, n_free], mybir.dt.float32, tag="ps", name="ps")
            for _ in range(N_WARM):
                nc.tensor.matmul(
                    ps_warm[:, :256],
                    warm_sb[:, :P],
                    warm_sb[:],
                    start=True,
                    stop=True,
                    skip_group_check=True,
                )

            # Chunk 0's weights are split per l-tile so the very first
            # matmuls gate on one l-half, not the full L-width chunk.
            w0_split = [
                wpool.tile(
                    [P, chunks[0][1], n_free], mybir.dt.float8e4, name=f"w0l{l}"
                )
                for l in range(NL)
            ]
            w_tiles = [None] + [
                wpool.tile([P, hi - lo, L], mybir.dt.float8e4, name=f"w{j}")
                for j, (lo, hi) in enumerate(chunks[1:], start=1)
            ]

            for i, (b0, bs) in enumerate(zip(offsets, slabs)):
                MSUB = bs // P
                if i == 0:
                    # Interleave resident-weight loads (sync queue) with
                    # slab-0 x loads (scalar HWDGE queue) in k-consumption
                    # order; two issue queues halve the descriptor-issue
                    # latency in front of the first matmul.
                    x_chunks = []
                    for j, (lo, hi) in enumerate(chunks):
                        if j == 0:
                            for l in range(NL):
                                nc.sync.dma_start(
                                    out=w0_split[l][:],
                                    in_=wT_r[:, lo:hi, l * n_free : (l + 1) * n_free],
                                )
                        else:
                            nc.sync.dma_start(
                                out=w_tiles[j][:], in_=wT_r[:, lo:hi, :]
                            )
                        xt = x0pool.tile(
                            [P, hi - lo, bs], mybir.dt.float8e4,
                            tag=f"x{j}", name=f"x{j}",
                        )
                        nc.scalar.dma_start(
                            out=xt[:], in_=xT_r[:, lo:hi, b0 : b0 + bs]
                        )
                        x_chunks.append(xt)
                else:
                    # Whole-slab X load; prefetch runs ~one slab (~55 us)
                    # ahead of consumption, so chunking buys nothing.
                    xs = xpool.tile(
                        [P, KSUB, bs], mybir.dt.float8e4, tag="xs", name="xs"
                    )
                    nc.scalar.dma_start(
                        out=xs[:], in_=xT_r[:, :, b0 : b0 + bs]
                    )

                def mm(ps, m, l, ks):
                    if i == 0:
                        ci, off = ks2chunk[ks]
                        xt = x_chunks[ci]
                        lhsT = xt[:, off : off + 2, m * P : (m + 1) * P]
                        if ci == 0:
                            rhs = w0_split[l][:, off : off + 2, :]
                        else:
                            rhs = w_tiles[ci][
                                :, off : off + 2, l * n_free : (l + 1) * n_free
                            ]
                    else:
                        ci, off = ks2chunk[ks]
                        lhsT = xs[:, ks : ks + 2, m * P : (m + 1) * P]
                        if ci == 0:
                            rhs = w0_split[l][:, off : off + 2, :]
                        else:
                            rhs = w_tiles[ci][
                                :, off : off + 2, l * n_free : (l + 1) * n_free
                            ]
                    nc.tensor.matmul(
                        ps[:],
                        lhsT,
                        rhs,
                        start=(ks == 0),
                        stop=(ks == KSUB - 2),
                        perf_mode=mybir.MatmulPerfMode.DoubleRow,
                        skip_group_check=True,
                    )

                def drain(ps, m, l):
                    ob = opool.tile([P, n_free], mybir.dt.uint8, tag="ob", name="ob")
                    nc.vector.tensor_scalar(
                        out=ob[:],
                        in0=ps[:],
                        scalar1=0.0,
                        scalar2=None,
                        op0=mybir.AluOpType.is_gt,
                    )
                    nc.sync.dma_start(
                        out=out[b0 + m * P : b0 + (m + 1) * P,
                                l * n_free : (l + 1) * n_free],
                        in_=ob[:],
                    )

                groups = [(m, l) for m in range(MSUB) for l in range(NL)]
                if i == 0 and len(groups) <= 8:
                    # Slab 0 is DMA-paced (the W+X broadcast is still in
                    # flight): run k OUTERMOST across all groups, one PSUM
                    # bank each, so every arriving k-chunk feeds 8x more PE
                    # work and the PE never outruns the DMA wave.
                    pss = {
                        g: pspool.tile(
                            [P, n_free], mybir.dt.float32, tag="ps", name="ps"
                        )
                        for g in groups
                    }
                    for ks in range(0, KSUB, 2):
                        for m, l in groups:
                            mm(pss[(m, l)], m, l, ks)
                    for m, l in groups:
                        drain(pss[(m, l)], m, l)
                else:
                    for m, l in groups:
                        ps = pspool.tile(
                            [P, n_free], mybir.dt.float32, tag="ps", name="ps"
                        )
                        for ks in range(0, KSUB, 2):
                            mm(ps, m, l, ks)
                        drain(ps, m, l)
    nc.compile()
    return nc


def to_fp8_bits(bool_arr_T):
    """bool/uint8 0-1 array -> fp8_e4m3 bytes holding 0.0 / 1.0 (0x38)."""
    a = np.ascontiguousarray(bool_arr_T).view(np.uint8) * np.uint8(0x38)
    return a.view(ml_dtypes.float8_e4m3)


_NC_CACHE = {}


def _get_nc(B, D, L):
    key = (B, D, L)
    if key not in _NC_CACHE:
        _NC_CACHE[key] = build_nc(B, D, L)
    return _NC_CACHE[key]


def run_spmd(x, bit_weights, trace=False, B=BATCH, D=IN_DIM, L_total=LAYER_SIZE):
    """Shared runner: returns (full bool output, BassKernelResults)."""
    n = N_CORES
    L = L_total // n
    nc = _get_nc(B, D, L)

    xT = to_fp8_bits(x.view(np.uint8).T)                      # (D, B)
    w_u8 = bit_weights.view(np.uint8)
    in_maps = []
    for m in range(n):
        wT_m = to_fp8_bits(w_u8[m * L : (m + 1) * L, :].T)    # (D, L)
        in_maps.append({"xT": xT, "wT": wT_m})

    res = run_bass_kernel_spmd(nc, in_maps, core_ids=list(range(n)), trace=trace)
    full = np.concatenate([res.results[m]["out"] for m in range(n)], axis=1)
    return full.view(np.bool_), res


def kernel(x, bit_weights):
    full, _ = run_spmd(np.asarray(x), np.asarray(bit_weights))
    return full


# revision 9
# speedup vs baseline: 1.0053x; 1.0053x over previous
"""Boolean OR-matmul kernel for Trainium2 (8 NeuronCores).

out[b, i] = OR_j (x[b, j] AND w[i, j])  ==  (x_f32 @ w.T_f32) > 0

Strategy:
- Shard bit_weights rows (layer_size 8192) across 8 cores -> 1024 rows/core,
  replicate x. No cross-core reduction needed; host concatenates column
  blocks of the output.
- Encode bools as fp8_e4m3 0.0/1.0 (bit pattern 0x38 == 1.0). Products are
  exactly 0/1, PSUM accumulates fp32 (counts <= 8192 < 2^24, exact), so
  (count > 0) is exact.
- Host pre-transposes both operands to put the contraction dim (in_features
  D) on the SBUF partition axis: xT (D, B), wT (D, Lshard). This makes every
  DMA a clean 2D/3D strided pattern with >=512B contiguous runs.
- PE does fp8 DoubleRow matmuls (K=256 per instruction), which the trace
  shows running at the true HW peak (216 ns per N=512 MM, LDWEIGHTS fully
  hidden). All remaining headroom is at the edges:
  * graduated fine-grained W/X chunks with W on the sync queue and X on the
    scalar-engine HWDGE queue, so the first matmul gates on ~200 KB instead
    of ~400 KB and slab-0 never outruns the DMA wave;
  * a short burst of dummy bf16 matmuls issued while the first chunks are
    still in flight pre-triggers the PE HAM clock-gate (cold 1.2 GHz ->
    warm 2.4 GHz) so the real stream starts warm;
  * slabs >= 1 load X as one whole-slab DMA (prefetch is ~50 us ahead).
- DVE thresholds PSUM fp32 -> uint8 0/1 via is_gt, DMA out.
"""

import sys

for _p in ("/opt/trn_rl_repo",):
    if _p not in sys.path:
        sys.path.insert(0, _p)

import numpy as np
import ml_dtypes

import concourse.bass as bass
import concourse.tile as tile
from concourse import bacc, mybir
from concourse.bass_utils import run_bass_kernel_spmd

P = 128          # SBUF partitions / PE contraction per k-subtile
N_CORES = 8

# Full problem shapes (hardcoded per harness contract)
BATCH = 4096
IN_DIM = 8192
LAYER_SIZE = 8192
L_SHARD = LAYER_SIZE // N_CORES  # 1024

N_WARM = 16      # dummy matmuls to pre-warm the PE HAM clock gate


def build_nc(B, D, L, b_slab=512, n_free=512):
    """Build the per-core Bass program.

    Per-core inputs : xT (D, B) fp8e4, wT (D, L) fp8e4
    Per-core output : out (B, L) uint8 (0/1)
    """
    assert D % (2 * P) == 0 and B % P == 0
    assert L % n_free == 0
    KSUB = D // P               # k-subtiles of 128
    NL = L // n_free            # l tiles
    assert B % b_slab == 0
    slabs = [b_slab] * (B // b_slab)
    offsets = [sum(slabs[:i]) for i in range(len(slabs))]

    nc = bacc.Bacc(None, target_bir_lowering=False, debug=False)
    xT = nc.dram_tensor("xT", [D, B], mybir.dt.float8e4, kind="ExternalInput")
    wT = nc.dram_tensor("wT", [D, L], mybir.dt.float8e4, kind="ExternalInput")
    out = nc.dram_tensor("out", [B, L], mybir.dt.uint8, kind="ExternalOutput")

    xT_r = xT.rearrange("(nk p) b -> p nk b", p=P)   # [128, KSUB, B]
    wT_r = wT.rearrange("(nk p) l -> p nk l", p=P)   # [128, KSUB, L]

    # Slab-0 is DMA-paced: chunk boundaries graduated so the first matmul
    # gates on a single k-subtile and the catch-up granularity stays fine
    # while the 12 MB W+X0 preload is in flight.
    bounds = sorted(
        {b for b in (0, 2, 4, 6, 8, 12, 16, 24, 32, 40, 48, 56) if b < KSUB}
        | {KSUB}
    )
    chunks = list(zip(bounds[:-1], bounds[1:]))  # [(lo, hi), ...]
    ks2chunk = {}
    for ci, (lo, hi) in enumerate(chunks):
        for ks in range(lo, hi):
            ks2chunk[ks] = (ci, ks - lo)

    with tile.TileContext(nc) as tc:
        with (
            tc.tile_pool(name="warm", bufs=1) as warmpool,
            tc.tile_pool(name="wpool", bufs=1) as wpool,
            tc.tile_pool(name="x0pool", bufs=1) as x0pool,
            tc.tile_pool(name="xpool", bufs=2) as xpool,
            tc.tile_pool(name="opool", bufs=4) as opool,
            tc.tile_pool(name="psum", bufs=8, space="PSUM") as pspool,
        ):
            # --- HAM pre-warm: cheap bf16 matmuls on a memset scratch tile
            # run while the first W/X chunks are still in flight, so the
            # PE clock gate opens before the real stream begins.
            warm_sb = warmpool.tile([P, 256], mybir.dt.bfloat16, name="warm")
            nc.gpsimd.memset(warm_sb[:], 0.0)
            ps_warm = pspool.tile([P, n_free], mybir.dt.float32, tag="ps", name="ps")
            for _ in range(N_WARM):
                nc.tensor.matmul(
                    ps_warm[:, :256],
                    warm_sb[:, :P],
                    warm_sb[:],
                    start=True,
                    stop=True,
                    skip_group_check=True,
                )

            # Chunk 0's weights are split per l-tile so the very first
            # matmuls gate on one l-half, not the full L-width chunk.
            w0_split = [
                wpool.tile(
                    [P, chunks[0][1], n_free], mybir.dt.float8e4, name=f"w0l{l}"
                )
                for l in range(NL)
            ]
            w_tiles = [None] + [
                wpool.tile([P, hi - lo, L], mybir.dt.float8e4, name=f"w{j}")
                for j, (lo, hi) in enumerate(chunks[1:], start=1)
            ]

            # --- Slab-0 front preload, strict consumption order on the
            # sync queue (w chunk k, then x chunk k, ...). The DMA issue
            # semaphore pool recycles every ~10 DMAs, so per-queue issue
            # order IS the transfer pacing order; consumption order keeps
            # the PE fed without later prefetches jumping the queue. Only
            # the first three (latency-critical) x chunks go on the scalar
            # HWDGE queue so the first matmul gates ~1 us earlier.
            b0_0, bs_0 = offsets[0], slabs[0]
            x_chunks = []
            for j, (lo, hi) in enumerate(chunks):
                if j == 0:
                    for l in range(NL):
                        nc.sync.dma_start(
                            out=w0_split[l][:],
                            in_=wT_r[:, lo:hi, l * n_free : (l + 1) * n_free],
                        )
                else:
                    nc.sync.dma_start(out=w_tiles[j][:], in_=wT_r[:, lo:hi, :])
                xt = x0pool.tile(
                    [P, hi - lo, bs_0], mybir.dt.float8e4,
                    tag=f"x{j}", name=f"x{j}",
                )
                eng = nc.scalar if j < 3 else nc.sync
                eng.dma_start(out=xt[:], in_=xT_r[:, lo:hi, b0_0 : b0_0 + bs_0])
                x_chunks.append(xt)

            xs_cur = None  # slab i's whole-slab x tile (i >= 1)

            for i, (b0, bs) in enumerate(zip(offsets, slabs)):
                MSUB = bs // P
                xs = xs_cur
                if i + 1 < len(slabs):
                    # Hoist slab-(i+1)'s whole-slab X prefetch BEFORE this
                    # slab's compute/drains: its issue then recycles only
                    # load-side semaphores, never an out-DMA semaphore
                    # (which would couple the prefetch to this slab's
                    # compute finishing — a ~2.6 us bubble per slab).
                    xs_cur = xpool.tile(
                        [P, KSUB, bs], mybir.dt.float8e4, tag="xs", name="xs"
                    )
                    nc.sync.dma_start(
                        out=xs_cur[:],
                        in_=xT_r[:, :, offsets[i + 1] : offsets[i + 1] + bs],
                    )

                def mm(ps, m, l, ks):
                    ci, off = ks2chunk[ks]
                    if i == 0:
                        xt = x_chunks[ci]
                        lhsT = xt[:, off : off + 2, m * P : (m + 1) * P]
                    else:
                        lhsT = xs[:, ks : ks + 2, m * P : (m + 1) * P]
                    if ci == 0:
                        rhs = w0_split[l][:, off : off + 2, :]
                    else:
                        rhs = w_tiles[ci][
                            :, off : off + 2, l * n_free : (l + 1) * n_free
                        ]
                    nc.tensor.matmul(
                        ps[:],
                        lhsT,
                        rhs,
                        start=(ks == 0),
                        stop=(ks == KSUB - 2),
                        perf_mode=mybir.MatmulPerfMode.DoubleRow,
                        skip_group_check=True,
                    )

                def drain(ps, m, l):
                    ob = opool.tile([P, n_free], mybir.dt.uint8, tag="ob", name="ob")
                    nc.vector.tensor_scalar(
                        out=ob[:],
                        in0=ps[:],
                        scalar1=0.0,
                        scalar2=None,
                        op0=mybir.AluOpType.is_gt,
                    )
                    nc.sync.dma_start(
                        out=out[b0 + m * P : b0 + (m + 1) * P,
                                l * n_free : (l + 1) * n_free],
                        in_=ob[:],
                    )

                groups = [(m, l) for m in range(MSUB) for l in range(NL)]
                if i == 0 and len(groups) <= 8:
                    # Slab 0 is DMA-paced (the W+X broadcast is still in
                    # flight): run k OUTERMOST across all groups, one PSUM
                    # bank each, so every arriving k-chunk feeds 8x more PE
                    # work and the PE never outruns the DMA wave.
                    pss = {
                        g: pspool.tile(
                            [P, n_free], mybir.dt.float32, tag="ps", name="ps"
                        )
                        for g in groups
                    }
                    for ks in range(0, KSUB, 2):
                        for m, l in groups:
                            mm(pss[(m, l)], m, l, ks)
                    for m, l in groups:
                        drain(pss[(m, l)], m, l)
                else:
                    for m, l in groups:
                        ps = pspool.tile(
                            [P, n_free], mybir.dt.float32, tag="ps", name="ps"
                        )
                        for ks in range(0, KSUB, 2):
                            mm(ps, m, l, ks)
                        drain(ps, m, l)
    nc.compile()
    return nc


def to_fp8_bits(bool_arr_T):
    """bool/uint8 0-1 array -> fp8_e4m3 bytes holding 0.0 / 1.0 (0x38)."""
    a = np.ascontiguousarray(bool_arr_T).view(np.uint8) * np.uint8(0x38)
    return a.view(ml_dtypes.float8_e4m3)


_NC_CACHE = {}


def _get_nc(B, D, L):
    key = (B, D, L)
    if key not in _NC_CACHE:
        _NC_CACHE[key] = build_nc(B, D, L)
    return _NC_CACHE[key]


def run_spmd(x, bit_weights, trace=False, B=BATCH, D=IN_DIM, L_total=LAYER_SIZE):
    """Shared runner: returns (full bool output, BassKernelResults)."""
    n = N_CORES
    L = L_total // n
    nc = _get_nc(B, D, L)

    xT = to_fp8_bits(x.view(np.uint8).T)                      # (D, B)
    w_u8 = bit_weights.view(np.uint8)
    in_maps = []
    for m in range(n):
        wT_m = to_fp8_bits(w_u8[m * L : (m + 1) * L, :].T)    # (D, L)
        in_maps.append({"xT": xT, "wT": wT_m})

    res = run_bass_kernel_spmd(nc, in_maps, core_ids=list(range(n)), trace=trace)
    full = np.concatenate([res.results[m]["out"] for m in range(n)], axis=1)
    return full.view(np.bool_), res


def kernel(x, bit_weights):
    full, _ = run_spmd(np.asarray(x), np.asarray(bit_weights))
    return full


# revision 10
# speedup vs baseline: 1.0081x; 1.0028x over previous
"""Boolean OR-matmul kernel for Trainium2 (8 NeuronCores).

out[b, i] = OR_j (x[b, j] AND w[i, j])  ==  (x_f32 @ w.T_f32) > 0

Strategy:
- Shard bit_weights rows (layer_size 8192) across 8 cores -> 1024 rows/core,
  replicate x. No cross-core reduction needed; host concatenates column
  blocks of the output.
- Encode bools as fp8_e4m3 0.0/1.0 (bit pattern 0x38 == 1.0). Products are
  exactly 0/1, PSUM accumulates fp32 (counts <= 8192 < 2^24, exact), so
  (count > 0) is exact.
- Host pre-transposes both operands to put the contraction dim (in_features
  D) on the SBUF partition axis: xT (D, B), wT (D, Lshard). This makes every
  DMA a clean 2D/3D strided pattern with >=512B contiguous runs.
- PE does fp8 DoubleRow matmuls (K=256 per instruction), which the trace
  shows running at the true HW peak (216 ns per N=512 MM, LDWEIGHTS fully
  hidden). All remaining headroom is at the edges:
  * graduated fine-grained W/X chunks with W on the sync queue and X on the
    scalar-engine HWDGE queue, so the first matmul gates on ~200 KB instead
    of ~400 KB and slab-0 never outruns the DMA wave;
  * a short burst of dummy bf16 matmuls issued while the first chunks are
    still in flight pre-triggers the PE HAM clock-gate (cold 1.2 GHz ->
    warm 2.4 GHz) so the real stream starts warm;
  * slabs >= 1 load X as one whole-slab DMA (prefetch is ~50 us ahead).
- DVE thresholds PSUM fp32 -> uint8 0/1 via is_gt, DMA out.
"""

import sys

for _p in ("/opt/trn_rl_repo",):
    if _p not in sys.path:
        sys.path.insert(0, _p)

import numpy as np
import ml_dtypes

import concourse.bass as bass
import concourse.tile as tile
from concourse import bacc, mybir
from concourse.bass_utils import run_bass_kernel_spmd

P = 128          # SBUF partitions / PE contraction per k-subtile
N_CORES = 8

# Full problem shapes (hardcoded per harness contract)
BATCH = 4096
IN_DIM = 8192
LAYER_SIZE = 8192
L_SHARD = LAYER_SIZE // N_CORES  # 1024

N_WARM = 16      # dummy matmuls to pre-warm the PE HAM clock gate


def build_nc(B, D, L, b_slab=512, n_free=512):
    """Build the per-core Bass program.

    Per-core inputs : xT (D, B) fp8e4, wT (D, L) fp8e4
    Per-core output : out (B, L) uint8 (0/1)
    """
    assert D % (2 * P) == 0 and B % P == 0
    assert L % n_free == 0
    KSUB = D // P               # k-subtiles of 128
    NL = L // n_free            # l tiles
    assert B % b_slab == 0
    slabs = [b_slab] * (B // b_slab)
    offsets = [sum(slabs[:i]) for i in range(len(slabs))]

    nc = bacc.Bacc(None, target_bir_lowering=False, debug=False)
    xT = nc.dram_tensor("xT", [D, B], mybir.dt.float8e4, kind="ExternalInput")
    wT = nc.dram_tensor("wT", [D, L], mybir.dt.float8e4, kind="ExternalInput")
    out = nc.dram_tensor("out", [B, L], mybir.dt.uint8, kind="ExternalOutput")

    xT_r = xT.rearrange("(nk p) b -> p nk b", p=P)   # [128, KSUB, B]
    wT_r = wT.rearrange("(nk p) l -> p nk l", p=P)   # [128, KSUB, L]

    # Slab-0 is DMA-paced: chunk boundaries graduated so the first matmul
    # gates on a single k-subtile and the catch-up granularity stays fine
    # while the 12 MB W+X0 preload is in flight.
    bounds = sorted(
        {b for b in (0, 2, 4, 6, 8, 12, 16, 24, 32, 40, 48, 56) if b < KSUB}
        | {KSUB}
    )
    chunks = list(zip(bounds[:-1], bounds[1:]))  # [(lo, hi), ...]
    ks2chunk = {}
    for ci, (lo, hi) in enumerate(chunks):
        for ks in range(lo, hi):
            ks2chunk[ks] = (ci, ks - lo)

    with tile.TileContext(nc) as tc:
        with (
            tc.tile_pool(name="warm", bufs=1) as warmpool,
            tc.tile_pool(name="wpool", bufs=1) as wpool,
            tc.tile_pool(name="x0pool", bufs=1) as x0pool,
            tc.tile_pool(name="xpool", bufs=2) as xpool,
            tc.tile_pool(name="opool", bufs=4) as opool,
            tc.tile_pool(name="psum", bufs=8, space="PSUM") as pspool,
        ):
            # --- HAM pre-warm: cheap bf16 matmuls on a memset scratch tile
            # run while the first W/X chunks are still in flight, so the
            # PE clock gate opens before the real stream begins.
            warm_sb = warmpool.tile([P, 256], mybir.dt.bfloat16, name="warm")
            nc.gpsimd.memset(warm_sb[:], 0.0)
            ps_warm = pspool.tile([P, n_free], mybir.dt.float32, tag="ps", name="ps")
            for _ in range(N_WARM):
                nc.tensor.matmul(
                    ps_warm[:, :256],
                    warm_sb[:, :P],
                    warm_sb[:],
                    start=True,
                    stop=True,
                    skip_group_check=True,
                )

            # Chunk 0's weights are split per l-tile so the very first
            # matmuls gate on one l-half, not the full L-width chunk.
            w0_split = [
                wpool.tile(
                    [P, chunks[0][1], n_free], mybir.dt.float8e4, name=f"w0l{l}"
                )
                for l in range(NL)
            ]
            w_tiles = [None] + [
                wpool.tile([P, hi - lo, L], mybir.dt.float8e4, name=f"w{j}")
                for j, (lo, hi) in enumerate(chunks[1:], start=1)
            ]

            # --- Slab-0 front preload, strict consumption order on the
            # sync queue (w chunk k, then x chunk k, ...). The DMA issue
            # semaphore pool recycles every ~10 DMAs, so per-queue issue
            # order IS the transfer pacing order; consumption order keeps
            # the PE fed without later prefetches jumping the queue. Only
            # the first three (latency-critical) x chunks go on the scalar
            # HWDGE queue so the first matmul gates ~1 us earlier.
            b0_0, bs_0 = offsets[0], slabs[0]
            x_chunks = []
            for j, (lo, hi) in enumerate(chunks):
                if j == 0:
                    for l in range(NL):
                        nc.sync.dma_start(
                            out=w0_split[l][:],
                            in_=wT_r[:, lo:hi, l * n_free : (l + 1) * n_free],
                        )
                else:
                    nc.sync.dma_start(out=w_tiles[j][:], in_=wT_r[:, lo:hi, :])
                xt = x0pool.tile(
                    [P, hi - lo, bs_0], mybir.dt.float8e4,
                    tag=f"x{j}", name=f"x{j}",
                )
                eng = nc.scalar if j < 3 else nc.sync
                eng.dma_start(out=xt[:], in_=xT_r[:, lo:hi, b0_0 : b0_0 + bs_0])
                x_chunks.append(xt)

            xs_cur = None  # slab i's whole-slab x tile (i >= 1)

            for i, (b0, bs) in enumerate(zip(offsets, slabs)):
                MSUB = bs // P
                xs = xs_cur
                if i + 1 < len(slabs):
                    # Hoist slab-(i+1)'s whole-slab X prefetch BEFORE this
                    # slab's compute/drains: its issue then recycles only
                    # load-side semaphores, never an out-DMA semaphore
                    # (which would couple the prefetch to this slab's
                    # compute finishing — a ~2.6 us bubble per slab).
                    xs_cur = xpool.tile(
                        [P, KSUB, bs], mybir.dt.float8e4, tag="xs", name="xs"
                    )
                    nc.sync.dma_start(
                        out=xs_cur[:],
                        in_=xT_r[:, :, offsets[i + 1] : offsets[i + 1] + bs],
                    )

                def mm(ps, m, l, ks):
                    ci, off = ks2chunk[ks]
                    if i == 0:
                        xt = x_chunks[ci]
                        lhsT = xt[:, off : off + 2, m * P : (m + 1) * P]
                    else:
                        lhsT = xs[:, ks : ks + 2, m * P : (m + 1) * P]
                    if ci == 0:
                        rhs = w0_split[l][:, off : off + 2, :]
                    else:
                        rhs = w_tiles[ci][
                            :, off : off + 2, l * n_free : (l + 1) * n_free
                        ]
                    nc.tensor.matmul(
                        ps[:],
                        lhsT,
                        rhs,
                        start=(ks == 0),
                        stop=(ks == KSUB - 2),
                        perf_mode=mybir.MatmulPerfMode.DoubleRow,
                        skip_group_check=True,
                    )

                def drain(ps, m, l):
                    ob = opool.tile([P, n_free], mybir.dt.uint8, tag="ob", name="ob")
                    nc.vector.tensor_scalar(
                        out=ob[:],
                        in0=ps[:],
                        scalar1=0.0,
                        scalar2=None,
                        op0=mybir.AluOpType.is_gt,
                    )
                    # Out-DMAs ride the scalar queue: it is idle after the
                    # front preload, so the issue instruction is pre-staged
                    # and fires the moment is_gt completes — and load-side
                    # semaphore recycling on sync never couples to them.
                    nc.scalar.dma_start(
                        out=out[b0 + m * P : b0 + (m + 1) * P,
                                l * n_free : (l + 1) * n_free],
                        in_=ob[:],
                    )

                groups = [(m, l) for m in range(MSUB) for l in range(NL)]
                if i == 0 and len(groups) <= 8:
                    # Slab 0 is DMA-paced (the W+X broadcast is still in
                    # flight): run k OUTERMOST across all groups, one PSUM
                    # bank each, so every arriving k-chunk feeds 8x more PE
                    # work and the PE never outruns the DMA wave.
                    pss = {
                        g: pspool.tile(
                            [P, n_free], mybir.dt.float32, tag="ps", name="ps"
                        )
                        for g in groups
                    }
                    for ks in range(0, KSUB, 2):
                        for m, l in groups:
                            mm(pss[(m, l)], m, l, ks)
                    for m, l in groups:
                        drain(pss[(m, l)], m, l)
                else:
                    for m, l in groups:
                        ps = pspool.tile(
                            [P, n_free], mybir.dt.float32, tag="ps", name="ps"
                        )
                        for ks in range(0, KSUB, 2):
                            mm(ps, m, l, ks)
                        drain(ps, m, l)
    nc.compile()
    return nc


def to_fp8_bits(bool_arr_T):
    """bool/uint8 0-1 array -> fp8_e4m3 bytes holding 0.0 / 1.0 (0x38)."""
    a = np.ascontiguousarray(bool_arr_T).view(np.uint8) * np.uint8(0x38)
    return a.view(ml_dtypes.float8_e4m3)


_NC_CACHE = {}


def _get_nc(B, D, L):
    key = (B, D, L)
    if key not in _NC_CACHE:
        _NC_CACHE[key] = build_nc(B, D, L)
    return _NC_CACHE[key]


def run_spmd(x, bit_weights, trace=False, B=BATCH, D=IN_DIM, L_total=LAYER_SIZE):
    """Shared runner: returns (full bool output, BassKernelResults)."""
    n = N_CORES
    L = L_total // n
    nc = _get_nc(B, D, L)

    xT = to_fp8_bits(x.view(np.uint8).T)                      # (D, B)
    w_u8 = bit_weights.view(np.uint8)
    in_maps = []
    for m in range(n):
        wT_m = to_fp8_bits(w_u8[m * L : (m + 1) * L, :].T)    # (D, L)
        in_maps.append({"xT": xT, "wT": wT_m})

    res = run_bass_kernel_spmd(nc, in_maps, core_ids=list(range(n)), trace=trace)
    full = np.concatenate([res.results[m]["out"] for m in range(n)], axis=1)
    return full.view(np.bool_), res


def kernel(x, bit_weights):
    full, _ = run_spmd(np.asarray(x), np.asarray(bit_weights))
    return full
